# revision 1
# baseline (speedup 1.0000x reference)
"""Trainium2 Bass kernel for nn_BasisDense: y = einsum('bd,duk,bk->bu', x, kernel, c_prob) + bias.

Strategy:
  - Factorize: t[b,(u,k)] = x @ kernel2d  (kernel2d = kernel.reshape(D, U*K), its
    NATURAL memory layout -> fully contiguous DMA of the 134MB kernel tensor),
    then y[b,u] = sum_k t[b,u,k]*c_prob[b,k] + bias[u] (cheap DVE epilogue).
  - Data-parallel shard batch B=4096 across 8 cores (512 rows each); kernel/bias
    replicated.
  - Matmuls run in float32r (full PE speed; ~1.5e-4 rms rel err vs fp32).
  - x is transposed on-device via PE transposes (lhsT needs [d, b] layout).
"""
import sys

sys.path.insert(0, "/opt/trn_rl_repo")

import numpy as np
import concourse.bacc as bacc
import concourse.mybir as mybir
import concourse.tile as tile
from concourse import bass_utils

B, D, U, K = 4096, 2048, 2048, 8
NCORES = 8
BS = B // NCORES  # 512 batch rows per core
UK = U * K  # 16384 fused (u,k) output columns
NFREE = 512  # matmul moving free dim (fp32 max, 1 PSUM bank)
NT = UK // NFREE  # 32 n-tiles
DT = D // 128  # 16 contraction tiles
BT = BS // 128  # 4 batch partition-tiles per core
UPT = NFREE // K  # 64 u-columns produced per n-tile

_CACHE = {}


def _build():
    nc = bacc.Bacc("TRN2", target_bir_lowering=False, debug=False, num_devices=NCORES)
    f32 = mybir.dt.float32
    f32r = mybir.dt.float32r

    x = nc.dram_tensor("x", [BS, D], f32, kind="ExternalInput").ap()
    cp = nc.dram_tensor("cp", [BS, K], f32, kind="ExternalInput").ap()
    kern = nc.dram_tensor("kern", [D, U, K], f32r, kind="ExternalInput").ap()
    bias = nc.dram_tensor("bias", [U], f32, kind="ExternalInput").ap()
    ident = nc.dram_tensor("ident", [128, 128], f32, kind="ExternalInput").ap()
    y = nc.dram_tensor("y", [BS, U], f32, kind="ExternalOutput").ap()

    # [128 d-partition, DT, UK] view of kernel2d
    kern2d = kern.rearrange("(t p) u k -> p t (u k)", p=128)

    with tile.TileContext(nc) as tc:
        with tc.tile_pool(name="const", bufs=1) as constp:
            id_sb = constp.tile([128, 128], f32)
            nc.sync.dma_start(id_sb, ident)

            # ---------------- setup: xT, c_rep, bias_rep ----------------
            xT = constp.tile([128, DT, BS], f32r)  # [d-part, d-tile, b]
            c_rep = constp.tile([128, BT, NFREE], f32)  # c_prob tiled 64x over u
            bias_rep = constp.tile([128, U], f32)  # bias bcast over partitions

            with (
                tc.tile_pool(name="setup_sb", bufs=2) as ssb,
                tc.tile_pool(name="setup_ps", bufs=2, space="PSUM") as sps,
            ):
                # x -> xT via PE transposes
                for bt in range(BT):
                    xa = ssb.tile([128, D], f32, tag="xa")
                    nc.sync.dma_start(xa, x[bt * 128 : (bt + 1) * 128, :])
                    for t in range(DT):
                        tp = sps.tile([128, 128], f32, tag="tp")
                        nc.tensor.transpose(tp, xa[:, t * 128 : (t + 1) * 128], id_sb)
                        # DVE copy rounds fp32 -> f32r (required by BIR verifier)
                        nc.vector.tensor_copy(xT[:, t, bt * 128 : (bt + 1) * 128], tp)

                # c_prob -> c_rep (replicate K-vector 64x along free dim)
                c_nat = ssb.tile([128, BT, K], f32, tag="cn")
                nc.sync.dma_start(c_nat, cp.rearrange("(bt p) k -> p bt k", p=128))
                for bt in range(BT):
                    nc.vector.tensor_copy(c_rep[:, bt, 0:K], c_nat[:, bt, :])
                    s = K
                    while s < NFREE:
                        nc.vector.tensor_copy(
                            c_rep[:, bt, s : 2 * s], c_rep[:, bt, 0:s]
                        )
                        s *= 2

                # bias -> bias_rep via ones-vector fp32 matmul broadcast
                bias_sb = ssb.tile([1, U], f32, tag="bs")
                nc.sync.dma_start(bias_sb, bias.unsqueeze(0))
                ones = ssb.tile([1, 128], f32, tag="ones")
                nc.vector.memset(ones, 1.0)
                for s in range(U // NFREE):
                    bps = sps.tile([128, NFREE], f32, tag="bps")
                    nc.tensor.matmul(
                        bps,
                        ones,
                        bias_sb[:, s * NFREE : (s + 1) * NFREE],
                        start=True,
                        stop=True,
                    )
                    nc.vector.tensor_copy(bias_rep[:, s * NFREE : (s + 1) * NFREE], bps)

            # ---------------- main loop ----------------
            with (
                tc.tile_pool(name="kt", bufs=3) as ktp,
                tc.tile_pool(name="mps", bufs=8, space="PSUM") as mps,
                tc.tile_pool(name="ep", bufs=4) as epp,
                tc.tile_pool(name="yp", bufs=8) as ypp,
            ):
                for n in range(NT):
                    kt = ktp.tile([128, DT, NFREE], f32r, tag="kt")
                    nc.sync.dma_start(
                        kt, kern2d[:, :, n * NFREE : (n + 1) * NFREE]
                    )
                    for bt in range(BT):
                        acc = mps.tile([128, NFREE], f32, tag="acc")
                        for t in range(DT):
                            nc.tensor.matmul(
                                acc,
                                xT[:, t, bt * 128 : (bt + 1) * 128],
                                kt[:, t, :],
                                start=(t == 0),
                                stop=(t == DT - 1),
                            )
                        # epilogue: y[b, u] = sum_k acc[b, (u,k)] * c[b, k] + bias[u]
                        tmp = epp.tile([128, NFREE], f32, tag="tmp")
                        nc.vector.tensor_mul(tmp, acc, c_rep[:, bt, :])
                        yt = ypp.tile([128, UPT], f32, tag="yt")
                        nc.vector.tensor_reduce(
                            yt,
                            tmp.rearrange("p (u k) -> p u k", k=K),
                            axis=mybir.AxisListType.X,
                            op=mybir.AluOpType.add,
                        )
                        yf = ypp.tile([128, UPT], f32, tag="yf")
                        nc.vector.tensor_add(
                            yf, yt, bias_rep[:, n * UPT : (n + 1) * UPT]
                        )
                        nc.sync.dma_start(
                            y[
                                bt * 128 : (bt + 1) * 128,
                                n * UPT : (n + 1) * UPT,
                            ],
                            yf,
                        )
    nc.compile()
    return nc


def kernel(x, c_prob, kernel, bias):
    if "nc" not in _CACHE:
        _CACHE["nc"] = _build()
    nc = _CACHE["nc"]
    x = np.ascontiguousarray(x, dtype=np.float32)
    c_prob = np.ascontiguousarray(c_prob, dtype=np.float32)
    kernel = np.ascontiguousarray(kernel, dtype=np.float32)
    bias = np.ascontiguousarray(bias, dtype=np.float32)
    ident = np.eye(128, dtype=np.float32)
    in_maps = [
        {
            "x": x[c * BS : (c + 1) * BS],
            "cp": c_prob[c * BS : (c + 1) * BS],
            "kern": kernel,
            "bias": bias,
            "ident": ident,
        }
        for c in range(NCORES)
    ]
    res = bass_utils.run_bass_kernel_spmd(nc, in_maps, list(range(NCORES)))
    return np.concatenate([res.results[c]["y"] for c in range(NCORES)], axis=0)


# revision 3
# speedup vs baseline: 1.0884x; 1.0884x over previous
"""Trainium2 Bass kernel for nn_BasisDense: y = einsum('bd,duk,bk->bu', x, kernel, c_prob) + bias.

Strategy:
  - Factorize: t[b,(u,k)] = x @ kernel2d  (kernel2d = kernel.reshape(D, U*K), its
    NATURAL memory layout -> fully contiguous DMA of the 134MB kernel tensor),
    then y[b,u] = sum_k t[b,u,k]*c_prob[b,k] + bias[u] (cheap DVE epilogue).
  - Data-parallel shard batch B=4096 across 8 cores (512 rows each); kernel/bias
    replicated.
  - Matmuls run in float32r (full PE speed; ~1.5e-4 rms rel err vs fp32).
  - x is transposed on-device via PE transposes (lhsT needs [d, b] layout).
"""
import sys

sys.path.insert(0, "/opt/trn_rl_repo")

import numpy as np
import concourse.bacc as bacc
import concourse.mybir as mybir
import concourse.tile as tile
from concourse import bass_utils

B, D, U, K = 4096, 2048, 2048, 8
NCORES = 8
BS = B // NCORES  # 512 batch rows per core
UK = U * K  # 16384 fused (u,k) output columns
NFREE = 512  # matmul moving free dim (fp32 max, 1 PSUM bank)
NT = UK // NFREE  # 32 n-tiles
DT = D // 128  # 16 contraction tiles
BT = BS // 128  # 4 batch partition-tiles per core
UPT = NFREE // K  # 64 u-columns produced per n-tile

_CACHE = {}


def _build():
    nc = bacc.Bacc("TRN2", target_bir_lowering=False, debug=False, num_devices=NCORES)
    f32 = mybir.dt.float32
    f32r = mybir.dt.float32r

    x = nc.dram_tensor("x", [BS, D], f32, kind="ExternalInput").ap()
    cp = nc.dram_tensor("cp", [BS, K], f32, kind="ExternalInput").ap()
    kern = nc.dram_tensor("kern", [D, U, K], f32r, kind="ExternalInput").ap()
    bias = nc.dram_tensor("bias", [U], f32, kind="ExternalInput").ap()
    ident = nc.dram_tensor("ident", [128, 128], f32, kind="ExternalInput").ap()
    y = nc.dram_tensor("y", [BS, U], f32, kind="ExternalOutput").ap()

    # [128 d-partition, DT, UK] view of kernel2d
    kern2d = kern.rearrange("(t p) u k -> p t (u k)", p=128)

    with tile.TileContext(nc) as tc:
        with tc.tile_pool(name="const", bufs=1) as constp:
            id_sb = constp.tile([128, 128], f32)
            nc.sync.dma_start(id_sb, ident)

            # ---------------- setup: xT, c_rep, bias_rep ----------------
            xT = constp.tile([128, DT, BS], f32r)  # [d-part, d-tile, b]
            c_rep = constp.tile([128, BT, NFREE], f32)  # c_prob tiled 64x over u
            bias_rep = constp.tile([128, U], f32)  # bias bcast over partitions

            with (
                tc.tile_pool(name="setup_sb", bufs=2) as ssb,
                tc.tile_pool(name="setup_ps", bufs=2, space="PSUM") as sps,
            ):
                # x -> xT via PE transposes
                for bt in range(BT):
                    xa = ssb.tile([128, D], f32, tag="xa")
                    nc.sync.dma_start(xa, x[bt * 128 : (bt + 1) * 128, :])
                    for t in range(DT):
                        tp = sps.tile([128, 128], f32, tag="tp")
                        nc.tensor.transpose(tp, xa[:, t * 128 : (t + 1) * 128], id_sb)
                        # DVE copy rounds fp32 -> f32r (required by BIR verifier)
                        nc.vector.tensor_copy(xT[:, t, bt * 128 : (bt + 1) * 128], tp)

                # c_prob -> c_rep (replicate K-vector 64x along free dim)
                c_nat = ssb.tile([128, BT, K], f32, tag="cn")
                nc.sync.dma_start(c_nat, cp.rearrange("(bt p) k -> p bt k", p=128))
                for bt in range(BT):
                    nc.vector.tensor_copy(c_rep[:, bt, 0:K], c_nat[:, bt, :])
                    s = K
                    while s < NFREE:
                        nc.vector.tensor_copy(
                            c_rep[:, bt, s : 2 * s], c_rep[:, bt, 0:s]
                        )
                        s *= 2

                # bias -> bias_rep via ones-vector fp32 matmul broadcast
                bias_sb = ssb.tile([1, U], f32, tag="bs")
                nc.sync.dma_start(bias_sb, bias.unsqueeze(0))
                ones = ssb.tile([1, 128], f32, tag="ones")
                nc.vector.memset(ones, 1.0)
                for s in range(U // NFREE):
                    bps = sps.tile([128, NFREE], f32, tag="bps")
                    nc.tensor.matmul(
                        bps,
                        ones,
                        bias_sb[:, s * NFREE : (s + 1) * NFREE],
                        start=True,
                        stop=True,
                    )
                    nc.vector.tensor_copy(bias_rep[:, s * NFREE : (s + 1) * NFREE], bps)

            # ---------------- main loop ----------------
            with (
                tc.tile_pool(name="kt", bufs=3) as ktp,
                tc.tile_pool(name="mps", bufs=8, space="PSUM") as mps,
                tc.tile_pool(name="ep", bufs=4) as epp,
                tc.tile_pool(name="yp", bufs=8) as ypp,
            ):
                for n in range(NT):
                    kt = ktp.tile([128, DT, NFREE], f32r, tag="kt")
                    # per-d-tile chunk DMAs (256KB each): the t-th matmul can
                    # start as soon as chunk t lands (subtile deps), instead of
                    # waiting for the whole 4MB tile
                    for t in range(DT):
                        nc.sync.dma_start(
                            kt[:, t, :],
                            kern2d[:, t, n * NFREE : (n + 1) * NFREE],
                        )
                    for bt in range(BT):
                        acc = mps.tile([128, NFREE], f32, tag="acc")
                        for t in range(DT):
                            nc.tensor.matmul(
                                acc,
                                xT[:, t, bt * 128 : (bt + 1) * 128],
                                kt[:, t, :],
                                start=(t == 0),
                                stop=(t == DT - 1),
                            )
                        # epilogue: y[b, u] = sum_k acc[b, (u,k)] * c[b, k] + bias[u]
                        tmp = epp.tile([128, NFREE], f32, tag="tmp")
                        nc.vector.tensor_mul(tmp, acc, c_rep[:, bt, :])
                        yt = ypp.tile([128, UPT], f32, tag="yt")
                        nc.vector.tensor_reduce(
                            yt,
                            tmp.rearrange("p (u k) -> p u k", k=K),
                            axis=mybir.AxisListType.X,
                            op=mybir.AluOpType.add,
                        )
                        yf = ypp.tile([128, UPT], f32, tag="yf")
                        nc.vector.tensor_add(
                            yf, yt, bias_rep[:, n * UPT : (n + 1) * UPT]
                        )
                        # route output DMAs through the idle scalar engine's
                        # HWDGE queue so they don't serialize behind kt chunks
                        nc.scalar.dma_start(
                            y[
                                bt * 128 : (bt + 1) * 128,
                                n * UPT : (n + 1) * UPT,
                            ],
                            yf,
                        )
    nc.compile()
    return nc


def kernel(x, c_prob, kernel, bias):
    if "nc" not in _CACHE:
        _CACHE["nc"] = _build()
    nc = _CACHE["nc"]
    x = np.ascontiguousarray(x, dtype=np.float32)
    c_prob = np.ascontiguousarray(c_prob, dtype=np.float32)
    kernel = np.ascontiguousarray(kernel, dtype=np.float32)
    bias = np.ascontiguousarray(bias, dtype=np.float32)
    ident = np.eye(128, dtype=np.float32)
    in_maps = [
        {
            "x": x[c * BS : (c + 1) * BS],
            "cp": c_prob[c * BS : (c + 1) * BS],
            "kern": kernel,
            "bias": bias,
            "ident": ident,
        }
        for c in range(NCORES)
    ]
    res = bass_utils.run_bass_kernel_spmd(nc, in_maps, list(range(NCORES)))
    return np.concatenate([res.results[c]["y"] for c in range(NCORES)], axis=0)


# revision 4
# speedup vs baseline: 1.0963x; 1.0072x over previous
"""Trainium2 Bass kernel for nn_BasisDense: y = einsum('bd,duk,bk->bu', x, kernel, c_prob) + bias.

Strategy:
  - Factorize: t[b,(u,k)] = x @ kernel2d  (kernel2d = kernel.reshape(D, U*K), its
    NATURAL memory layout -> fully contiguous DMA of the 134MB kernel tensor),
    then y[b,u] = sum_k t[b,u,k]*c_prob[b,k] + bias[u] (cheap DVE epilogue).
  - Data-parallel shard batch B=4096 across 8 cores (512 rows each); kernel/bias
    replicated.
  - Matmuls run in float32r (full PE speed; ~1.5e-4 rms rel err vs fp32).
  - x is transposed on-device via PE transposes (lhsT needs [d, b] layout).
"""
import sys

sys.path.insert(0, "/opt/trn_rl_repo")

import numpy as np
import concourse.bacc as bacc
import concourse.mybir as mybir
import concourse.tile as tile
from concourse import bass_utils

B, D, U, K = 4096, 2048, 2048, 8
NCORES = 8
BS = B // NCORES  # 512 batch rows per core
UK = U * K  # 16384 fused (u,k) output columns
NFREE = 512  # matmul moving free dim (fp32 max, 1 PSUM bank)
NT = UK // NFREE  # 32 n-tiles
DT = D // 128  # 16 contraction tiles
BT = BS // 128  # 4 batch partition-tiles per core
UPT = NFREE // K  # 64 u-columns produced per n-tile

_CACHE = {}


def _build():
    nc = bacc.Bacc("TRN2", target_bir_lowering=False, debug=False, num_devices=NCORES)
    f32 = mybir.dt.float32
    f32r = mybir.dt.float32r

    x = nc.dram_tensor("x", [BS, D], f32, kind="ExternalInput").ap()
    cp = nc.dram_tensor("cp", [BS, K], f32, kind="ExternalInput").ap()
    kern = nc.dram_tensor("kern", [D, U, K], f32r, kind="ExternalInput").ap()
    bias = nc.dram_tensor("bias", [U], f32, kind="ExternalInput").ap()
    ident = nc.dram_tensor("ident", [128, 128], f32, kind="ExternalInput").ap()
    y = nc.dram_tensor("y", [BS, U], f32, kind="ExternalOutput").ap()

    # [128 d-partition, DT, UK] view of kernel2d
    kern2d = kern.rearrange("(t p) u k -> p t (u k)", p=128)

    with tile.TileContext(nc) as tc:
        with tc.tile_pool(name="const", bufs=1) as constp:
            id_sb = constp.tile([128, 128], f32)
            nc.scalar.dma_start(id_sb, ident)

            # ---------------- setup: xT, c_rep, bias_rep ----------------
            xT = constp.tile([128, DT, BS], f32r)  # [d-part, d-tile, b]
            c_rep = constp.tile([128, BT, NFREE], f32)  # c_prob tiled 64x over u
            bias_rep = constp.tile([128, U], f32)  # bias bcast over partitions

            with (
                tc.tile_pool(name="setup_sb", bufs=2) as ssb,
                tc.tile_pool(name="setup_ps", bufs=2, space="PSUM") as sps,
            ):
                # x -> xT via PE transposes
                for bt in range(BT):
                    xa = ssb.tile([128, D], f32, tag="xa")
                    # bt=0 on the sync queue (critical path); rest on scalar so
                    # the first kernel-chunk DMAs aren't stuck behind 3MB of x
                    eng = nc.sync if bt == 0 else nc.scalar
                    eng.dma_start(xa, x[bt * 128 : (bt + 1) * 128, :])
                    for t in range(DT):
                        tp = sps.tile([128, 128], f32, tag="tp")
                        nc.tensor.transpose(tp, xa[:, t * 128 : (t + 1) * 128], id_sb)
                        # DVE copy rounds fp32 -> f32r (required by BIR verifier)
                        nc.vector.tensor_copy(xT[:, t, bt * 128 : (bt + 1) * 128], tp)

                # c_prob -> c_rep (replicate K-vector 64x along free dim)
                c_nat = ssb.tile([128, BT, K], f32, tag="cn")
                nc.scalar.dma_start(c_nat, cp.rearrange("(bt p) k -> p bt k", p=128))
                for bt in range(BT):
                    nc.vector.tensor_copy(c_rep[:, bt, 0:K], c_nat[:, bt, :])
                    s = K
                    while s < NFREE:
                        nc.vector.tensor_copy(
                            c_rep[:, bt, s : 2 * s], c_rep[:, bt, 0:s]
                        )
                        s *= 2

                # bias -> bias_rep via ones-vector fp32 matmul broadcast
                bias_sb = ssb.tile([1, U], f32, tag="bs")
                nc.scalar.dma_start(bias_sb, bias.unsqueeze(0))
                ones = ssb.tile([1, 128], f32, tag="ones")
                nc.vector.memset(ones, 1.0)
                for s in range(U // NFREE):
                    bps = sps.tile([128, NFREE], f32, tag="bps")
                    nc.tensor.matmul(
                        bps,
                        ones,
                        bias_sb[:, s * NFREE : (s + 1) * NFREE],
                        start=True,
                        stop=True,
                    )
                    nc.vector.tensor_copy(bias_rep[:, s * NFREE : (s + 1) * NFREE], bps)

            # ---------------- main loop ----------------
            with (
                tc.tile_pool(name="kt", bufs=4) as ktp,
                tc.tile_pool(name="mps", bufs=8, space="PSUM") as mps,
                tc.tile_pool(name="ep", bufs=4) as epp,
                tc.tile_pool(name="yp", bufs=8) as ypp,
            ):
                for n in range(NT):
                    kt = ktp.tile([128, DT, NFREE], f32r, tag="kt")
                    # per-d-tile chunk DMAs (256KB each): the t-th matmul can
                    # start as soon as chunk t lands (subtile deps), instead of
                    # waiting for the whole 4MB tile
                    for t in range(DT):
                        eng = nc.sync if t % 2 == 0 else nc.scalar
                        eng.dma_start(
                            kt[:, t, :],
                            kern2d[:, t, n * NFREE : (n + 1) * NFREE],
                        )
                    for bt in range(BT):
                        acc = mps.tile([128, NFREE], f32, tag="acc")
                        for t in range(DT):
                            nc.tensor.matmul(
                                acc,
                                xT[:, t, bt * 128 : (bt + 1) * 128],
                                kt[:, t, :],
                                start=(t == 0),
                                stop=(t == DT - 1),
                            )
                        # epilogue: y[b, u] = sum_k acc[b, (u,k)] * c[b, k] + bias[u]
                        tmp = epp.tile([128, NFREE], f32, tag="tmp")
                        nc.vector.tensor_mul(tmp, acc, c_rep[:, bt, :])
                        yt = ypp.tile([128, UPT], f32, tag="yt")
                        nc.vector.tensor_reduce(
                            yt,
                            tmp.rearrange("p (u k) -> p u k", k=K),
                            axis=mybir.AxisListType.X,
                            op=mybir.AluOpType.add,
                        )
                        yf = ypp.tile([128, UPT], f32, tag="yf")
                        nc.vector.tensor_add(
                            yf, yt, bias_rep[:, n * UPT : (n + 1) * UPT]
                        )
                        # route output DMAs through the idle scalar engine's
                        # HWDGE queue so they don't serialize behind kt chunks
                        nc.scalar.dma_start(
                            y[
                                bt * 128 : (bt + 1) * 128,
                                n * UPT : (n + 1) * UPT,
                            ],
                            yf,
                        )
    nc.compile()
    return nc


def kernel(x, c_prob, kernel, bias):
    if "nc" not in _CACHE:
        _CACHE["nc"] = _build()
    nc = _CACHE["nc"]
    x = np.ascontiguousarray(x, dtype=np.float32)
    c_prob = np.ascontiguousarray(c_prob, dtype=np.float32)
    kernel = np.ascontiguousarray(kernel, dtype=np.float32)
    bias = np.ascontiguousarray(bias, dtype=np.float32)
    ident = np.eye(128, dtype=np.float32)
    in_maps = [
        {
            "x": x[c * BS : (c + 1) * BS],
            "cp": c_prob[c * BS : (c + 1) * BS],
            "kern": kernel,
            "bias": bias,
            "ident": ident,
        }
        for c in range(NCORES)
    ]
    res = bass_utils.run_bass_kernel_spmd(nc, in_maps, list(range(NCORES)))
    return np.concatenate([res.results[c]["y"] for c in range(NCORES)], axis=0)


# revision 5
# speedup vs baseline: 1.1371x; 1.0372x over previous
"""Trainium2 Bass kernel for nn_BasisDense: y = einsum('bd,duk,bk->bu', x, kernel, c_prob) + bias.

Strategy:
  - Factorize: t[b,(u,k)] = x @ kernel2d  (kernel2d = kernel.reshape(D, U*K), its
    NATURAL memory layout -> fully contiguous DMA of the 134MB kernel tensor),
    then y[b,u] = sum_k t[b,u,k]*c_prob[b,k] + bias[u] (cheap DVE epilogue).
  - Data-parallel shard batch B=4096 across 8 cores (512 rows each); kernel/bias
    replicated.
  - Matmuls run in float32r (full PE speed; ~1.5e-4 rms rel err vs fp32).
  - Host-side input marshaling: x transposed to [D, BS] (lhsT layout), c_prob
    tiled to the (u,k)-interleaved epilogue layout, bias broadcast over the 128
    partitions. All O(B*D + U) work, negligible vs the O(B*D*U*K) kernel.
"""
import sys

sys.path.insert(0, "/opt/trn_rl_repo")

import numpy as np
import concourse.bacc as bacc
import concourse.mybir as mybir
import concourse.tile as tile
from concourse import bass_utils

B, D, U, K = 4096, 2048, 2048, 8
NCORES = 8
BS = B // NCORES  # 512 batch rows per core
UK = U * K  # 16384 fused (u,k) output columns
NFREE = 512  # matmul moving free dim (fp32 max, 1 PSUM bank)
NT = UK // NFREE  # 32 n-tiles
DT = D // 128  # 16 contraction tiles
BT = BS // 128  # 4 batch partition-tiles per core
UPT = NFREE // K  # 64 u-columns produced per n-tile

_CACHE = {}


def _build():
    nc = bacc.Bacc("TRN2", target_bir_lowering=False, debug=False, num_devices=NCORES)
    f32 = mybir.dt.float32
    f32r = mybir.dt.float32r

    xt = nc.dram_tensor("xt", [D, BS], f32r, kind="ExternalInput").ap()
    crep = nc.dram_tensor("crep", [BS, NFREE], f32, kind="ExternalInput").ap()
    kern = nc.dram_tensor("kern", [D, U, K], f32r, kind="ExternalInput").ap()
    biasr = nc.dram_tensor("biasr", [128, U], f32, kind="ExternalInput").ap()
    y = nc.dram_tensor("y", [BS, U], f32, kind="ExternalOutput").ap()

    # [128 d-partition, DT, UK] view of kernel2d
    kern2d = kern.rearrange("(t p) u k -> p t (u k)", p=128)

    with tile.TileContext(nc) as tc:
        with (
            tc.tile_pool(name="const", bufs=1) as constp,
            tc.tile_pool(name="kt", bufs=4) as ktp,
            tc.tile_pool(name="mps", bufs=8, space="PSUM") as mps,
            tc.tile_pool(name="ep", bufs=4) as epp,
            tc.tile_pool(name="yp", bufs=8) as ypp,
        ):
            xT = constp.tile([128, DT, BS], f32r)  # [d-part, d-tile, b]
            c_rep = constp.tile([128, BT, NFREE], f32)
            bias_rep = constp.tile([128, U], f32)

            # xT chunks on the sync queue ahead of the kernel stream; the
            # first matmul only needs chunk t=0 (subtile deps)
            xt_v = xt.rearrange("(t p) b -> p t b", p=128)
            for t in range(DT):
                nc.sync.dma_start(xT[:, t, :], xt_v[:, t, :])
            nc.scalar.dma_start(c_rep, crep.rearrange("(bt p) n -> p bt n", p=128))
            nc.scalar.dma_start(bias_rep, biasr)

            for n in range(NT):
                kt = ktp.tile([128, DT, NFREE], f32r, tag="kt")
                # per-d-tile chunk DMAs (256KB each): the t-th matmul can start
                # as soon as chunk t lands, alternating across two HWDGE queues
                for t in range(DT):
                    eng = nc.sync if t % 2 == 0 else nc.scalar
                    eng.dma_start(
                        kt[:, t, :],
                        kern2d[:, t, n * NFREE : (n + 1) * NFREE],
                    )
                for bt in range(BT):
                    acc = mps.tile([128, NFREE], f32, tag="acc")
                    for t in range(DT):
                        nc.tensor.matmul(
                            acc,
                            xT[:, t, bt * 128 : (bt + 1) * 128],
                            kt[:, t, :],
                            start=(t == 0),
                            stop=(t == DT - 1),
                        )
                    # epilogue: y[b, u] = sum_k acc[b, (u,k)] * c[b, k] + bias[u]
                    tmp = epp.tile([128, NFREE], f32, tag="tmp")
                    nc.vector.tensor_mul(tmp, acc, c_rep[:, bt, :])
                    yt = ypp.tile([128, UPT], f32, tag="yt")
                    nc.vector.tensor_reduce(
                        yt,
                        tmp.rearrange("p (u k) -> p u k", k=K),
                        axis=mybir.AxisListType.X,
                        op=mybir.AluOpType.add,
                    )
                    yf = ypp.tile([128, UPT], f32, tag="yf")
                    nc.vector.tensor_add(yf, yt, bias_rep[:, n * UPT : (n + 1) * UPT])
                    # output DMAs ride the scalar engine's HWDGE queue
                    nc.scalar.dma_start(
                        y[bt * 128 : (bt + 1) * 128, n * UPT : (n + 1) * UPT],
                        yf,
                    )
    nc.compile()
    return nc


def _in_maps(x, c_prob, kernel, bias):
    x = np.ascontiguousarray(x, dtype=np.float32)
    c_prob = np.ascontiguousarray(c_prob, dtype=np.float32)
    kernel = np.ascontiguousarray(kernel, dtype=np.float32)
    bias = np.ascontiguousarray(bias, dtype=np.float32)
    bias_rep = np.ascontiguousarray(np.broadcast_to(bias, (128, U)))
    maps = []
    for c in range(NCORES):
        xs = x[c * BS : (c + 1) * BS]
        cs = c_prob[c * BS : (c + 1) * BS]
        maps.append(
            {
                "xt": np.ascontiguousarray(xs.T),
                "crep": np.ascontiguousarray(np.tile(cs, (1, UPT))),
                "kern": kernel,
                "biasr": bias_rep,
            }
        )
    return maps


def kernel(x, c_prob, kernel, bias):
    if "nc" not in _CACHE:
        _CACHE["nc"] = _build()
    nc = _CACHE["nc"]
    res = bass_utils.run_bass_kernel_spmd(
        nc, _in_maps(x, c_prob, kernel, bias), list(range(NCORES))
    )
    return np.concatenate([res.results[c]["y"] for c in range(NCORES)], axis=0)
